# revision 10
# baseline (speedup 1.0000x reference)
"""Bass/Trainium2 kernel for nn_Channel_attention (bottom-16 channel gather).

reference semantics (per sample b):
    weight = mean(x[b], axis=(H, W))           # [C]
    idx    = argsort(weight)[:16]              # ascending pooled value
    out[b] = x[b, idx]                         # [16, H, W]

Strategy: pure data parallel, B=16 sharded 2 samples per core over 8 cores.
Per core (x shard viewed as [512, 16384] = [(sample, channel), H*W]):
  1. Stream each half-sample (128 channels x 16384) into one of three
     resident SBUF regions (ring; 24 MiB of SBUF), loads alternating
     between the sync and scalar HWDGE queues.  DVE reduce_add per
     2048-wide chunk -> per-channel partial sums.
  2. Per sample: negate sums, PE-transpose into a [1, 256] row, two
     rounds of max8/max_index/match_replace -> bottom-16 channel
     indices in ascending order of pooled sum.
  3. Invert the permutation on-chip: per half, dest[p] = s*16 + rank(p)
     if channel p is selected else OOB.  One SWDGE indirect *scatter*
     per half writes the 16 selected channel rows straight from the
     resident SBUF region to the output (bounds_check + oob skip drops
     the other 112 rows).  No HBM gather re-read, no store pass.
"""

import sys

if "/opt/trn_rl_repo" not in sys.path:
    sys.path.insert(0, "/opt/trn_rl_repo")

import numpy as np

from concourse import bacc, mybir, tile
from concourse.bass import IndirectOffsetOnAxis
from concourse.bass_utils import run_bass_kernel_spmd
from concourse.masks import make_identity

N_CORES = 8
B, C, H, W = 16, 256, 128, 128
K = 16
BPC = B // N_CORES          # samples per core = 2
E = H * W                   # 16384 elems per channel
ROWS = BPC * C              # 512 channel rows per core
NREG = 3                    # resident half-sample regions (ring)
OOB = 1.0e6                 # scatter destination for non-selected rows

f32 = mybir.dt.float32
i32 = mybir.dt.int32
u32 = mybir.dt.uint32
X = mybir.AxisListType.X
Alu = mybir.AluOpType

# chunk widths per (sample, half); the last half ends with small chunks so
# the final reduce exits quickly after the last load lands
CHUNKS = [2048] * 8
CHUNKS_LAST = [2048] * 5 + [1024] * 6

_cache = {}


def _build():
    nc = bacc.Bacc("TRN2", target_bir_lowering=False, debug=False,
                   num_devices=N_CORES)
    x_d = nc.dram_tensor("x", [ROWS, E], f32, kind="ExternalInput")
    y_d = nc.dram_tensor("y", [BPC * K, E], f32, kind="ExternalOutput")

    with tile.TileContext(nc) as tc:
        with (
            tc.tile_pool(name="reg", bufs=1) as reg_pool,
            tc.tile_pool(name="small", bufs=1) as small,
            tc.tile_pool(name="psum", bufs=1, space="PSUM") as psum,
        ):
            # ---- constants (no deps; scheduler fills gaps with these) ----
            ident = small.tile([128, 128], f32)
            make_identity(nc, ident[:])

            kio = small.tile([128, K], i32)   # value k in column k, all parts
            nc.gpsimd.iota(out=kio[:], pattern=[[1, K]], base=0,
                           channel_multiplier=0)
            kio_f = small.tile([128, K], f32)
            nc.vector.tensor_copy(kio_f[:], kio[:])

            chan = small.tile([128, 1], i32)  # partition index
            nc.gpsimd.iota(out=chan[:], pattern=[[1, 1]], base=0,
                           channel_multiplier=1)
            chan_f = [small.tile([128, 1], f32, tag=f"chan{h}",
                                 name=f"chan{h}")
                      for h in range(2)]
            nc.vector.tensor_copy(chan_f[0][:], chan[:])
            nc.vector.tensor_scalar(out=chan_f[1][:], in0=chan_f[0][:],
                                    scalar1=128.0, scalar2=None, op0=Alu.add)

            regions = [reg_pool.tile([128, E], f32, tag=f"R{i}",
                                     name=f"R{i}")
                       for i in range(NREG)]

            dma_engines = [nc.sync, nc.scalar]
            n_dma = 0

            # ---- per-sample pipeline ----
            for s in range(BPC):
                chunk_lists = [CHUNKS if not (s == BPC - 1 and h == 1)
                               else CHUNKS_LAST for h in range(2)]
                ncols = max(len(cl) for cl in chunk_lists)
                partials = small.tile([128, 2 * ncols], f32,
                                      tag=f"partials{s}")
                sums = small.tile([128, 2], f32, tag=f"sums{s}")
                psum_w = psum.tile([1, C], f32, tag=f"psw{s}")
                w_neg = small.tile([1, C], f32, tag=f"wneg{s}")

                for h in range(2):
                    R = regions[(2 * s + h) % NREG]
                    base = s * C + h * 128
                    off = 0
                    cl = chunk_lists[h]
                    for j, cw in enumerate(cl):
                        eng = dma_engines[n_dma % 2]
                        n_dma += 1
                        eng.dma_start(out=R[:, off:off + cw],
                                      in_=x_d[base:base + 128, off:off + cw])
                        nc.vector.reduce_sum(
                            out=partials[:, h * ncols + j:h * ncols + j + 1],
                            in_=R[:, off:off + cw], axis=X)
                        off += cw

                    # this half's sums + transpose, while the other half
                    # (or the next sample) is still streaming; keep DVE's
                    # stream clear of everything but the chunk reduces
                    dump = small.tile([128, ncols], f32, tag=f"dump{s}_{h}")
                    nc.scalar.activation(
                        out=dump[:, 0:len(cl)],
                        in_=partials[:, h * ncols:h * ncols + len(cl)],
                        func=mybir.ActivationFunctionType.Copy,
                        scale=-1.0, accum_out=sums[:, h:h + 1])
                    nc.tensor.matmul(out=psum_w[:, h * 128:(h + 1) * 128],
                                     lhsT=sums[:, h:h + 1], rhs=ident[:],
                                     start=True, stop=True)
                    nc.scalar.copy(w_neg[:, h * 128:(h + 1) * 128],
                                   psum_w[:, h * 128:(h + 1) * 128])

                # bottom-16 via two rounds of max8 on -sums
                m1 = small.tile([1, 8], f32, tag=f"m1_{s}")
                m2 = small.tile([1, 8], f32, tag=f"m2_{s}")
                idx_u = small.tile([1, K], u32, tag=f"idxu{s}")
                w_rep = small.tile([1, C], f32, tag=f"wrep{s}")

                nc.vector.max(out=m1[:], in_=w_neg[:])
                nc.vector.max_index(out=idx_u[:, 0:8], in_max=m1[:],
                                    in_values=w_neg[:])
                nc.vector.match_replace(out=w_rep[:], in_to_replace=m1[:],
                                        in_values=w_neg[:], imm_value=-1e38)
                nc.vector.max(out=m2[:], in_=w_rep[:])
                nc.vector.max_index(out=idx_u[:, 8:16], in_max=m2[:],
                                    in_values=w_rep[:])

                # broadcast idx row to all 128 partitions (gpsimd)
                idx_f = small.tile([1, K], f32, tag=f"idxf{s}")
                nc.vector.tensor_copy(idx_f[:], idx_u[:])
                idx_bc = small.tile([128, K], f32, tag=f"idxbc{s}")
                nc.gpsimd.partition_broadcast(idx_bc[:], idx_f[:])

                # per half: dest[p] = s*K + rank  (selected) else OOB,
                # then indirect-scatter the 16 selected rows out of SBUF
                for h in range(2):
                    R = regions[(2 * s + h) % NREG]
                    eq = small.tile([128, K], f32, tag=f"eq{s}_{h}")
                    nc.gpsimd.tensor_scalar(out=eq[:], in0=idx_bc[:],
                                            scalar1=chan_f[h][:],
                                            scalar2=None, op0=Alu.is_equal)
                    rkv = small.tile([128, K], f32, tag=f"rkv{s}_{h}")
                    nc.gpsimd.tensor_tensor(out=rkv[:], in0=eq[:],
                                            in1=kio_f[:], op=Alu.mult)
                    rk = small.tile([128, 1], f32, tag=f"rk{s}_{h}")
                    rkd = small.tile([128, K], f32, tag=f"rkd{s}_{h}")
                    nc.scalar.activation(
                        out=rkd[:], in_=rkv[:],
                        func=mybir.ActivationFunctionType.Copy,
                        accum_out=rk[:])
                    msk = small.tile([128, 1], f32, tag=f"msk{s}_{h}")
                    mkd = small.tile([128, K], f32, tag=f"mkd{s}_{h}")
                    nc.scalar.activation(
                        out=mkd[:], in_=eq[:],
                        func=mybir.ActivationFunctionType.Copy,
                        accum_out=msk[:])
                    dst_f = small.tile([128, 1], f32, tag=f"dstf{s}_{h}")
                    nc.gpsimd.tensor_scalar(out=dst_f[:], in0=msk[:],
                                            scalar1=-OOB,
                                            scalar2=OOB + s * K,
                                            op0=Alu.mult, op1=Alu.add)
                    nc.gpsimd.tensor_tensor(out=dst_f[:], in0=rk[:],
                                            in1=dst_f[:], op=Alu.add)
                    dst_i = small.tile([128, 1], i32, tag=f"dsti{s}_{h}")
                    nc.gpsimd.tensor_copy(dst_i[:], dst_f[:])

                    nc.gpsimd.indirect_dma_start(
                        out=y_d[:],
                        out_offset=IndirectOffsetOnAxis(ap=dst_i[:], axis=0),
                        in_=R[:, :], in_offset=None,
                        bounds_check=BPC * K - 1, oob_is_err=False)

    nc.compile()
    return nc


def get_nc():
    if "nc" not in _cache:
        _cache["nc"] = _build()
    return _cache["nc"]


def make_in_maps(x: np.ndarray) -> list[dict[str, np.ndarray]]:
    x = np.ascontiguousarray(np.asarray(x, dtype=np.float32))
    assert x.shape == (B, C, H, W)
    return [{"x": x[c * BPC:(c + 1) * BPC].reshape(ROWS, E)}
            for c in range(N_CORES)]


def assemble(results: list[dict[str, np.ndarray]]) -> np.ndarray:
    out = np.empty((B, K, H, W), dtype=np.float32)
    for c in range(N_CORES):
        out[c * BPC:(c + 1) * BPC] = results[c]["y"].reshape(BPC, K, H, W)
    return out


def kernel(x: np.ndarray) -> np.ndarray:
    nc = get_nc()
    res = run_bass_kernel_spmd(nc, make_in_maps(x), list(range(N_CORES)))
    return assemble(res.results)


# revision 11
# speedup vs baseline: 1.1886x; 1.1886x over previous
"""Bass/Trainium2 kernel for nn_Channel_attention (bottom-16 channel gather).

reference semantics (per sample b):
    weight = mean(x[b], axis=(H, W))           # [C]
    idx    = argsort(weight)[:16]              # ascending pooled value
    out[b] = x[b, idx]                         # [16, H, W]

Strategy: pure data parallel, B=16 sharded 2 samples per core over 8 cores.
Per core (x shard viewed as [512, 16384] = [(sample, channel), H*W]):
  1. Stream each half-sample (128 channels x 16384) into one of three
     resident SBUF regions (ring; 24 MiB of SBUF), loads alternating
     between the sync and scalar HWDGE queues.  Per-chunk partial sums
     alternate between DVE reduce_add and ACT activation-accumulate
     (ACT dumps its elementwise output into PSUM) so neither engine
     becomes the streaming bottleneck.
  2. Per sample: ACT second-level reduce (negated), PE-transpose into a
     [1, 256] row, two rounds of max8/max_index/match_replace on DVE ->
     bottom-16 channel indices in ascending order of pooled sum.
  3. Invert the permutation on-chip (PE broadcast + small DVE ops):
     per half, dest[p] = s*16 + rank(p) if channel p is selected else
     OOB.  One SWDGE indirect *scatter* per half writes the 16 selected
     channel rows straight from the resident SBUF region to the output
     (bounds_check + oob skip drops the other 112 rows).  No HBM gather
     re-read, no separate store pass.
"""

import sys

if "/opt/trn_rl_repo" not in sys.path:
    sys.path.insert(0, "/opt/trn_rl_repo")

import numpy as np

from concourse import bacc, mybir, tile
from concourse.bass import IndirectOffsetOnAxis
from concourse.bass_utils import run_bass_kernel_spmd
from concourse.masks import make_identity

N_CORES = 8
B, C, H, W = 16, 256, 128, 128
K = 16
BPC = B // N_CORES          # samples per core = 2
E = H * W                   # 16384 elems per channel
ROWS = BPC * C              # 512 channel rows per core
NREG = 3                    # resident half-sample regions (ring)
OOB = 1.0e6                 # scatter destination for non-selected rows

f32 = mybir.dt.float32
i32 = mybir.dt.int32
u32 = mybir.dt.uint32
X = mybir.AxisListType.X
Alu = mybir.AluOpType
ActCopy = mybir.ActivationFunctionType.Copy

# chunk widths per (sample, half); the last half ends with small chunks so
# the final reduce exits quickly after the last load lands
CHUNKS = [2048] * 8
CHUNKS_LAST = [2048] * 5 + [1024] * 6

_cache = {}


def _build():
    nc = bacc.Bacc("TRN2", target_bir_lowering=False, debug=False,
                   num_devices=N_CORES)
    x_d = nc.dram_tensor("x", [ROWS, E], f32, kind="ExternalInput")
    y_d = nc.dram_tensor("y", [BPC * K, E], f32, kind="ExternalOutput")

    with tile.TileContext(nc) as tc:
        with (
            tc.tile_pool(name="reg", bufs=1) as reg_pool,
            tc.tile_pool(name="small", bufs=1) as small,
            tc.tile_pool(name="psum", bufs=1, space="PSUM") as psum,
        ):
            # ---- constants (no deps; scheduler fills gaps with these) ----
            ident = small.tile([128, 128], f32)
            make_identity(nc, ident[:])

            kio = small.tile([128, K], i32)   # value k in column k, all parts
            nc.gpsimd.iota(out=kio[:], pattern=[[1, K]], base=0,
                           channel_multiplier=0)
            kio_f = small.tile([128, K], f32)
            nc.vector.tensor_copy(kio_f[:], kio[:])

            chan = small.tile([128, 1], i32)  # partition index
            nc.gpsimd.iota(out=chan[:], pattern=[[1, 1]], base=0,
                           channel_multiplier=1)
            chan_f = [small.tile([128, 1], f32, tag=f"chan{h}",
                                 name=f"chan{h}")
                      for h in range(2)]
            nc.vector.tensor_copy(chan_f[0][:], chan[:])
            nc.vector.tensor_scalar(out=chan_f[1][:], in0=chan_f[0][:],
                                    scalar1=128.0, scalar2=None, op0=Alu.add)

            ones_row = small.tile([1, 128], f32)
            nc.vector.memset(ones_row[:], 1.0)

            regions = [reg_pool.tile([128, E], f32, tag=f"R{i}",
                                     name=f"R{i}")
                       for i in range(NREG)]
            # ACT chunk-reduce elementwise output goes to PSUM (discarded)
            psdump = psum.tile([128, 2048], f32, tag="psdump")

            dma_engines = [nc.sync, nc.scalar]
            n_dma = 0

            halves = [(s, h) for s in range(BPC) for h in range(2)]
            chunk_list = {(s, h): (CHUNKS_LAST
                                   if (s == BPC - 1 and h == 1) else CHUNKS)
                          for (s, h) in halves}
            ncols = max(len(cl) for cl in chunk_list.values())
            partials = [small.tile([128, 2 * ncols], f32, tag=f"partials{s}",
                                   name=f"partials{s}")
                        for s in range(BPC)]
            offs = {}
            for key, cl in chunk_list.items():
                o, acc = [], 0
                for cw in cl:
                    o.append(acc)
                    acc += cw
                offs[key] = o

            def emit_loads(s, h):
                nonlocal n_dma
                R = regions[(2 * s + h) % NREG]
                base = s * C + h * 128
                for j, cw in enumerate(chunk_list[(s, h)]):
                    off = offs[(s, h)][j]
                    eng = dma_engines[n_dma % 2]
                    n_dma += 1
                    eng.dma_start(out=R[:, off:off + cw],
                                  in_=x_d[base:base + 128, off:off + cw])

            def emit_reduces(s, h):
                R = regions[(2 * s + h) % NREG]
                P = partials[s]
                for j, cw in enumerate(chunk_list[(s, h)]):
                    off = offs[(s, h)][j]
                    col = h * ncols + j
                    if j % 2 == 0:
                        nc.vector.reduce_sum(out=P[:, col:col + 1],
                                             in_=R[:, off:off + cw], axis=X)
                    else:
                        nc.scalar.activation(out=psdump[:, 0:cw],
                                             in_=R[:, off:off + cw],
                                             func=ActCopy,
                                             accum_out=P[:, col:col + 1])

            def emit_finalize(s):
                P = partials[s]
                sums = small.tile([128, 2], f32, tag=f"sums{s}",
                                  name=f"sums{s}")
                psum_w = psum.tile([1, C], f32, tag=f"psw{s}",
                                   name=f"psw{s}")
                w_neg = small.tile([1, C], f32, tag=f"wneg{s}",
                                   name=f"wneg{s}")
                for h in range(2):
                    cl = chunk_list[(s, h)]
                    dump = small.tile([128, ncols], f32, tag=f"dump{s}_{h}",
                                      name=f"dump{s}_{h}")
                    nc.scalar.activation(
                        out=dump[:, 0:len(cl)],
                        in_=P[:, h * ncols:h * ncols + len(cl)],
                        func=ActCopy, scale=-1.0,
                        accum_out=sums[:, h:h + 1])
                    nc.tensor.matmul(out=psum_w[:, h * 128:(h + 1) * 128],
                                     lhsT=sums[:, h:h + 1], rhs=ident[:],
                                     start=True, stop=True)
                    nc.scalar.copy(w_neg[:, h * 128:(h + 1) * 128],
                                   psum_w[:, h * 128:(h + 1) * 128])

                # bottom-16 via two rounds of max8 on -sums (DVE)
                m1 = small.tile([1, 8], f32, tag=f"m1_{s}", name=f"m1_{s}")
                m2 = small.tile([1, 8], f32, tag=f"m2_{s}", name=f"m2_{s}")
                idx_u = small.tile([1, K], u32, tag=f"idxu{s}",
                                   name=f"idxu{s}")
                w_rep = small.tile([1, C], f32, tag=f"wrep{s}",
                                   name=f"wrep{s}")
                nc.vector.max(out=m1[:], in_=w_neg[:])
                nc.vector.max_index(out=idx_u[:, 0:8], in_max=m1[:],
                                    in_values=w_neg[:])
                nc.vector.match_replace(out=w_rep[:], in_to_replace=m1[:],
                                        in_values=w_neg[:], imm_value=-1e38)
                nc.vector.max(out=m2[:], in_=w_rep[:])
                nc.vector.max_index(out=idx_u[:, 8:16], in_max=m2[:],
                                    in_values=w_rep[:])

                # broadcast idx row to all 128 partitions via PE
                idx_f = small.tile([1, K], f32, tag=f"idxf{s}",
                                   name=f"idxf{s}")
                nc.vector.tensor_copy(idx_f[:], idx_u[:])
                psum_b = psum.tile([128, K], f32, tag=f"psb{s}",
                                   name=f"psb{s}")
                nc.tensor.matmul(out=psum_b[:], lhsT=ones_row[:],
                                 rhs=idx_f[:], start=True, stop=True)
                idx_bc = small.tile([128, K], f32, tag=f"idxbc{s}",
                                    name=f"idxbc{s}")
                nc.scalar.copy(idx_bc[:], psum_b[:])

                # per half: dest[p] = s*K + rank (selected) else OOB (DVE),
                # then indirect-scatter the selected rows out of SBUF
                for h in range(2):
                    R = regions[(2 * s + h) % NREG]
                    eq = small.tile([128, K], f32, tag=f"eq{s}_{h}",
                                    name=f"eq{s}_{h}")
                    nc.vector.tensor_scalar(out=eq[:], in0=idx_bc[:],
                                            scalar1=chan_f[h][:],
                                            scalar2=None, op0=Alu.is_equal)
                    rkv = small.tile([128, K], f32, tag=f"rkv{s}_{h}",
                                     name=f"rkv{s}_{h}")
                    nc.vector.tensor_tensor(out=rkv[:], in0=eq[:],
                                            in1=kio_f[:], op=Alu.mult)
                    rk = small.tile([128, 1], f32, tag=f"rk{s}_{h}",
                                    name=f"rk{s}_{h}")
                    nc.vector.reduce_sum(out=rk[:], in_=rkv[:], axis=X)
                    msk = small.tile([128, 1], f32, tag=f"msk{s}_{h}",
                                     name=f"msk{s}_{h}")
                    nc.vector.reduce_sum(out=msk[:], in_=eq[:], axis=X)
                    dst_f = small.tile([128, 1], f32, tag=f"dstf{s}_{h}",
                                       name=f"dstf{s}_{h}")
                    nc.vector.tensor_scalar(out=dst_f[:], in0=msk[:],
                                            scalar1=-OOB,
                                            scalar2=OOB + s * K,
                                            op0=Alu.mult, op1=Alu.add)
                    nc.vector.tensor_tensor(out=dst_f[:], in0=rk[:],
                                            in1=dst_f[:], op=Alu.add)
                    dst_i = small.tile([128, 1], i32, tag=f"dsti{s}_{h}",
                                       name=f"dsti{s}_{h}")
                    nc.vector.tensor_copy(dst_i[:], dst_f[:])

                    nc.gpsimd.indirect_dma_start(
                        out=y_d[:],
                        out_offset=IndirectOffsetOnAxis(ap=dst_i[:], axis=0),
                        in_=R[:, :], in_offset=None,
                        bounds_check=BPC * K - 1, oob_is_err=False)

            # emission order keeps each in-order queue free of blocking
            # waits: sample 1's last-half loads (which reuse R0) are
            # emitted only after sample 0's scatters
            emit_loads(0, 0)
            emit_loads(0, 1)
            emit_reduces(0, 0)
            emit_loads(1, 0)
            emit_reduces(0, 1)
            emit_finalize(0)
            emit_loads(1, 1)
            emit_reduces(1, 0)
            emit_reduces(1, 1)
            emit_finalize(1)

    nc.compile()
    return nc


def get_nc():
    if "nc" not in _cache:
        _cache["nc"] = _build()
    return _cache["nc"]


def make_in_maps(x: np.ndarray) -> list[dict[str, np.ndarray]]:
    x = np.ascontiguousarray(np.asarray(x, dtype=np.float32))
    assert x.shape == (B, C, H, W)
    return [{"x": x[c * BPC:(c + 1) * BPC].reshape(ROWS, E)}
            for c in range(N_CORES)]


def assemble(results: list[dict[str, np.ndarray]]) -> np.ndarray:
    out = np.empty((B, K, H, W), dtype=np.float32)
    for c in range(N_CORES):
        out[c * BPC:(c + 1) * BPC] = results[c]["y"].reshape(BPC, K, H, W)
    return out


def kernel(x: np.ndarray) -> np.ndarray:
    nc = get_nc()
    res = run_bass_kernel_spmd(nc, make_in_maps(x), list(range(N_CORES)))
    return assemble(res.results)


# revision 17
# speedup vs baseline: 1.3572x; 1.1419x over previous
"""Bass/Trainium2 kernel for nn_Channel_attention (bottom-16 channel gather).

reference semantics (per sample b):
    weight = mean(x[b], axis=(H, W))           # [C]
    idx    = argsort(weight)[:16]              # ascending pooled value
    out[b] = x[b, idx]                         # [16, H, W]

Strategy: pure data parallel, B=16 sharded 2 samples per core over 8 cores.
Per core (x shard viewed as [512, 16384] = [(sample, channel), H*W]):
  1. Stream each half-sample (128 channels x 16384) into one of three
     resident SBUF regions (ring; 24 MiB of SBUF).  ALL load DMAs go on
     the sync HWDGE queue (one InstDMACopy already spreads over all 16
     SDMA engines), leaving the scalar/ACT queue free for compute.
  2. Per-chunk partial sums alternate between DVE reduce_add and ACT
     activation-accumulate (ACT dumps its elementwise output to PSUM);
     either engine alone is slightly slower than the DMA feed, together
     they have 2x headroom.  ACT does the negated second-level reduce,
     PE transposes the per-channel sums into a [1, 256] row, and two
     rounds of max8/max_index/match_replace on DVE give the bottom-16
     channel indices in ascending order of pooled sum.
  3. Invert the permutation on-chip (PE broadcast + small DVE ops):
     per half, dest[p] = s*16 + rank(p) if channel p is selected else
     OOB.  One SWDGE indirect *scatter* per half writes the selected
     channel rows straight from the resident SBUF region to that
     half's own output tensor (separate tensors break the WAW chain so
     the two scatters overlap; bounds_check + oob skip drops the other
     rows).  The bottom-16 indices are also written out so the host
     can assemble rows from the right half.  No HBM gather re-read.
"""

import sys

if "/opt/trn_rl_repo" not in sys.path:
    sys.path.insert(0, "/opt/trn_rl_repo")

import numpy as np

from concourse import bacc, mybir, tile
from concourse.bass import IndirectOffsetOnAxis
from concourse.bass_utils import run_bass_kernel_spmd
from concourse.masks import make_identity

N_CORES = 8
B, C, H, W = 16, 256, 128, 128
K = 16
BPC = B // N_CORES          # samples per core = 2
E = H * W                   # 16384 elems per channel
ROWS = BPC * C              # 512 channel rows per core
NREG = 3                    # resident half-sample regions (ring)
OOB = 1.0e6                 # scatter destination for non-selected rows

f32 = mybir.dt.float32
i32 = mybir.dt.int32
u32 = mybir.dt.uint32
X = mybir.AxisListType.X
Alu = mybir.AluOpType
ActCopy = mybir.ActivationFunctionType.Copy

# chunk widths per (sample, half); the last half ends with small chunks so
# the final reduce exits quickly after the last load lands
CHUNKS = [2048] * 8
CHUNKS_LAST = [2048] * 5 + [1024] * 6

_cache = {}


def _build():
    nc = bacc.Bacc("TRN2", target_bir_lowering=False, debug=False,
                   num_devices=N_CORES)
    x_d = nc.dram_tensor("x", [ROWS, E], f32, kind="ExternalInput")
    y_h = [nc.dram_tensor(f"y{h}", [BPC * K, E], f32, kind="ExternalOutput")
           for h in range(2)]
    idx_d = nc.dram_tensor("idx", [BPC, K], u32, kind="ExternalOutput")

    with tile.TileContext(nc) as tc:
        with (
            tc.tile_pool(name="reg", bufs=1) as reg_pool,
            tc.tile_pool(name="small", bufs=1) as small,
            tc.tile_pool(name="psum", bufs=1, space="PSUM") as psum,
        ):
            # ---- constants (no deps; scheduler fills gaps with these) ----
            ident = small.tile([128, 128], f32)
            make_identity(nc, ident[:])

            kio = small.tile([128, K], i32)   # value k in column k, all parts
            nc.gpsimd.iota(out=kio[:], pattern=[[1, K]], base=0,
                           channel_multiplier=0)
            kio_f = small.tile([128, K], f32)
            nc.vector.tensor_copy(kio_f[:], kio[:])

            chan = small.tile([128, 1], i32)  # partition index
            nc.gpsimd.iota(out=chan[:], pattern=[[1, 1]], base=0,
                           channel_multiplier=1)
            chan_f = [small.tile([128, 1], f32, tag=f"chan{h}",
                                 name=f"chan{h}")
                      for h in range(2)]
            nc.vector.tensor_copy(chan_f[0][:], chan[:])
            nc.vector.tensor_scalar(out=chan_f[1][:], in0=chan_f[0][:],
                                    scalar1=128.0, scalar2=None, op0=Alu.add)

            ones_row = small.tile([1, 128], f32)
            nc.vector.memset(ones_row[:], 1.0)

            regions = [reg_pool.tile([128, E], f32, tag=f"R{i}",
                                     name=f"R{i}")
                       for i in range(NREG)]
            # ACT chunk-reduce elementwise output goes to PSUM (discarded)
            psdump = psum.tile([128, 2048], f32, tag="psdump")

            halves = [(s, h) for s in range(BPC) for h in range(2)]
            chunk_list = {(s, h): (CHUNKS_LAST
                                   if (s == BPC - 1 and h == 1) else CHUNKS)
                          for (s, h) in halves}
            ncols = max(len(cl) for cl in chunk_list.values())
            partials = [small.tile([128, 2 * ncols], f32, tag=f"partials{s}",
                                   name=f"partials{s}")
                        for s in range(BPC)]
            offs = {}
            for key, cl in chunk_list.items():
                o, acc = [], 0
                for cw in cl:
                    o.append(acc)
                    acc += cw
                offs[key] = o

            def emit_loads(s, h):
                R = regions[(2 * s + h) % NREG]
                base = s * C + h * 128
                for j, cw in enumerate(chunk_list[(s, h)]):
                    off = offs[(s, h)][j]
                    nc.sync.dma_start(out=R[:, off:off + cw],
                                      in_=x_d[base:base + 128, off:off + cw])

            def emit_reduces(s, h):
                R = regions[(2 * s + h) % NREG]
                P = partials[s]
                for j, cw in enumerate(chunk_list[(s, h)]):
                    off = offs[(s, h)][j]
                    col = h * ncols + j
                    if j % 2 == 0:
                        nc.vector.reduce_sum(out=P[:, col:col + 1],
                                             in_=R[:, off:off + cw], axis=X)
                    else:
                        nc.scalar.activation(out=psdump[:, 0:cw],
                                             in_=R[:, off:off + cw],
                                             func=ActCopy,
                                             accum_out=P[:, col:col + 1])

            def emit_finalize(s):
                P = partials[s]
                sums = small.tile([128, 2], f32, tag=f"sums{s}",
                                  name=f"sums{s}")
                psum_w = psum.tile([1, C], f32, tag=f"psw{s}",
                                   name=f"psw{s}")
                w_neg = small.tile([1, C], f32, tag=f"wneg{s}",
                                   name=f"wneg{s}")
                for h in range(2):
                    cl = chunk_list[(s, h)]
                    dump = small.tile([128, ncols], f32, tag=f"dump{s}_{h}",
                                      name=f"dump{s}_{h}")
                    nc.scalar.activation(
                        out=dump[:, 0:len(cl)],
                        in_=P[:, h * ncols:h * ncols + len(cl)],
                        func=ActCopy, scale=-1.0,
                        accum_out=sums[:, h:h + 1])
                    nc.tensor.matmul(out=psum_w[:, h * 128:(h + 1) * 128],
                                     lhsT=sums[:, h:h + 1], rhs=ident[:],
                                     start=True, stop=True)
                    nc.scalar.copy(w_neg[:, h * 128:(h + 1) * 128],
                                   psum_w[:, h * 128:(h + 1) * 128])

                # bottom-16 via two rounds of max8 on -sums (DVE)
                m1 = small.tile([1, 8], f32, tag=f"m1_{s}", name=f"m1_{s}")
                m2 = small.tile([1, 8], f32, tag=f"m2_{s}", name=f"m2_{s}")
                idx_u = small.tile([1, K], u32, tag=f"idxu{s}",
                                   name=f"idxu{s}")
                w_rep = small.tile([1, C], f32, tag=f"wrep{s}",
                                   name=f"wrep{s}")
                nc.vector.max(out=m1[:], in_=w_neg[:])
                nc.vector.max_index(out=idx_u[:, 0:8], in_max=m1[:],
                                    in_values=w_neg[:])
                nc.vector.match_replace(out=w_rep[:], in_to_replace=m1[:],
                                        in_values=w_neg[:], imm_value=-1e38)
                nc.vector.max(out=m2[:], in_=w_rep[:])
                nc.vector.max_index(out=idx_u[:, 8:16], in_max=m2[:],
                                    in_values=w_rep[:])
                nc.scalar.dma_start(out=idx_d[s:s + 1, :], in_=idx_u[:])

                # broadcast idx row to all 128 partitions via PE
                idx_f = small.tile([1, K], f32, tag=f"idxf{s}",
                                   name=f"idxf{s}")
                nc.vector.tensor_copy(idx_f[:], idx_u[:])
                psum_b = psum.tile([128, K], f32, tag=f"psb{s}",
                                   name=f"psb{s}")
                nc.tensor.matmul(out=psum_b[:], lhsT=ones_row[:],
                                 rhs=idx_f[:], start=True, stop=True)
                idx_bc = small.tile([128, K], f32, tag=f"idxbc{s}",
                                    name=f"idxbc{s}")
                nc.vector.tensor_copy(idx_bc[:], psum_b[:])

                # per half: dest[p] = s*K + rank (selected) else OOB (DVE),
                # then indirect-scatter the selected rows out of SBUF into
                # that half's own output tensor
                for h in range(2):
                    R = regions[(2 * s + h) % NREG]
                    eq = small.tile([128, K], f32, tag=f"eq{s}_{h}",
                                    name=f"eq{s}_{h}")
                    nc.vector.tensor_scalar(out=eq[:], in0=idx_bc[:],
                                            scalar1=chan_f[h][:],
                                            scalar2=None, op0=Alu.is_equal)
                    rkv = small.tile([128, K], f32, tag=f"rkv{s}_{h}",
                                     name=f"rkv{s}_{h}")
                    nc.vector.tensor_tensor(out=rkv[:], in0=eq[:],
                                            in1=kio_f[:], op=Alu.mult)
                    rk = small.tile([128, 1], f32, tag=f"rk{s}_{h}",
                                    name=f"rk{s}_{h}")
                    nc.vector.reduce_sum(out=rk[:], in_=rkv[:], axis=X)
                    msk = small.tile([128, 1], f32, tag=f"msk{s}_{h}",
                                     name=f"msk{s}_{h}")
                    nc.vector.reduce_sum(out=msk[:], in_=eq[:], axis=X)
                    dst_f = small.tile([128, 1], f32, tag=f"dstf{s}_{h}",
                                       name=f"dstf{s}_{h}")
                    nc.vector.tensor_scalar(out=dst_f[:], in0=msk[:],
                                            scalar1=-OOB,
                                            scalar2=OOB + s * K,
                                            op0=Alu.mult, op1=Alu.add)
                    nc.vector.tensor_tensor(out=dst_f[:], in0=rk[:],
                                            in1=dst_f[:], op=Alu.add)
                    dst_i = small.tile([128, 1], i32, tag=f"dsti{s}_{h}",
                                       name=f"dsti{s}_{h}")
                    nc.vector.tensor_copy(dst_i[:], dst_f[:])

                    nc.gpsimd.indirect_dma_start(
                        out=y_h[h][:],
                        out_offset=IndirectOffsetOnAxis(ap=dst_i[:], axis=0),
                        in_=R[:, :], in_offset=None,
                        bounds_check=BPC * K - 1, oob_is_err=False)

            # emission order keeps each in-order queue free of blocking
            # waits: sample 1's last-half loads (which reuse R0) are
            # emitted only after sample 0's scatters
            emit_loads(0, 0)
            emit_loads(0, 1)
            emit_reduces(0, 0)
            emit_loads(1, 0)
            emit_reduces(0, 1)
            emit_finalize(0)
            emit_loads(1, 1)
            emit_reduces(1, 0)
            emit_reduces(1, 1)
            emit_finalize(1)

    nc.compile()
    return nc


def get_nc():
    if "nc" not in _cache:
        _cache["nc"] = _build()
    return _cache["nc"]


def make_in_maps(x: np.ndarray) -> list[dict[str, np.ndarray]]:
    x = np.ascontiguousarray(np.asarray(x, dtype=np.float32))
    assert x.shape == (B, C, H, W)
    return [{"x": x[c * BPC:(c + 1) * BPC].reshape(ROWS, E)}
            for c in range(N_CORES)]


def assemble(results: list[dict[str, np.ndarray]]) -> np.ndarray:
    out = np.empty((B, K, H, W), dtype=np.float32)
    for c in range(N_CORES):
        idx = results[c]["idx"]                     # [BPC, K] u32
        y0 = results[c]["y0"].reshape(BPC, K, H, W)
        y1 = results[c]["y1"].reshape(BPC, K, H, W)
        for s in range(BPC):
            hsel = (idx[s] // 128)[:, None, None]   # [K, 1, 1]
            out[c * BPC + s] = np.where(hsel == 0, y0[s], y1[s])
    return out


def kernel(x: np.ndarray) -> np.ndarray:
    nc = get_nc()
    res = run_bass_kernel_spmd(nc, make_in_maps(x), list(range(N_CORES)))
    return assemble(res.results)
